# revision 16
# baseline (speedup 1.0000x reference)
"""Binary-weight 3x3 conv (stride 1, pad 1) on 8 TRN2 NeuronCores.

Strategy: data-parallel over batch (4 images per core), weights replicated.
Per image the conv is 9 shifted [Cin,Cout] matmuls accumulated in PSUM
with channels on the partition dim (NCHW layout already has x[n] as a
[C, H*W] channel-major matrix). The input lives in SBUF as fp32r rows of
width 57: data cols 0..55 plus one zero column that doubles as the next
row's LEFT pad, with zero rows above/below (flat [1 + 58*57 + 1] layout).
Every matmul rhs is then a fully CONTIGUOUS 1D window of N=456 covering
8 output rows (one junk psum column per row, discarded by the PSUM->SBUF
copy). Contiguous 1D rhs APs issue ~7ns/matmul faster than 3D strided
ones. fp32r (reduced-precision fp32 matmul mode) runs at 1 cycle/row on
the PE, 4x faster than plain fp32; binarized weights (+-1, 0) are exact.
Input is DMA'd contiguously into a staging tile (halves split across the
sync+scalar HWDGE queues) and padded/rounded into fp32r by a DVE copy.
Each PSUM bank gets its 18 accumulating matmuls back-to-back and drains
immediately; the first group runs cin-tile-major for a 63-matmul runway
while the second cin tile's DMA completes, and the first weight block is
DMA'd per-tap so tap 0 lands before the image does.
"""

import numpy as np

N_CORES = 8
B_PER_CORE = 4  # 32 images / 8 cores
CIN = 256
COUT = 256
H = W = 56
WR = 57  # row pitch: 56 data + 1 shared pad col
XLEN = 1 + 58 * WR + 1  # leading pad slot + 58 rows + trailing slot
RB = 8  # output rows per matmul
NBLK = H // RB  # 7
NFREE = RB * WR  # 456 (8 rows x 57, one junk col per row)

_CACHED = {}


def _build_nc():
    import concourse.mybir as mybir
    from concourse import bacc
    from concourse.tile import TileContext

    f32 = mybir.dt.float32
    f32r = mybir.dt.float32r

    nc = bacc.Bacc("TRN2", target_bir_lowering=False, debug=False)
    xs = nc.dram_tensor("xs", [B_PER_CORE, CIN, H, W], f32, kind="ExternalInput").ap()
    wt = nc.dram_tensor("wt", [4, 128, 9, 128], f32, kind="ExternalInput").ap()
    out = nc.dram_tensor(
        "out", [B_PER_CORE, COUT, H, W], f32, kind="ExternalOutput"
    ).ap()

    with TileContext(nc) as tc:
        with (
            tc.tile_pool(name="wp", bufs=1) as wp,
            tc.tile_pool(name="sp", bufs=2) as sp,
            tc.tile_pool(name="xp", bufs=8) as xp,
            tc.tile_pool(name="yp", bufs=16) as yp,
            tc.tile_pool(name="pp", bufs=8, space="PSUM") as pp,
        ):
            # fp32r memset is not a legal ISA op; round zeros in via DVE copy
            zrow = wp.tile([128, 58], f32, name="zrow")
            nc.vector.memset(zrow[:], 0.0)

            w_sb = wp.tile([128, 4, 9, 128], f32r, name="w_sb")

            HH = H // 2
            xt = {}

            def load_image(n):
                for cit in range(2):
                    stage = sp.tile([128, H, W], f32, name="stage", tag="stage")
                    src = xs[n, cit * 128 : (cit + 1) * 128]
                    if n == 0 and cit == 0:
                        # first weight tap leads on scalar so the very first
                        # matmul is unblocked before the image lands
                        nc.scalar.dma_start(
                            out=w_sb[:, 0, 0], in_=wt[0, :, 0].bitcast(f32r)
                        )
                    nc.sync.dma_start(out=stage[:, :HH], in_=src[:, :HH])
                    nc.scalar.dma_start(out=stage[:, HH:], in_=src[:, HH:])
                    if n == 0 and cit == 0:
                        for k in range(1, 9):
                            nc.sync.dma_start(
                                out=w_sb[:, 0, k], in_=wt[0, :, k].bitcast(f32r)
                            )
                        nc.scalar.dma_start(out=w_sb[:, 1], in_=wt[1].bitcast(f32r))
                    if n == 0 and cit == 1:
                        nc.sync.dma_start(out=w_sb[:, 2], in_=wt[2].bitcast(f32r))
                        nc.scalar.dma_start(out=w_sb[:, 3], in_=wt[3].bitcast(f32r))
                    t = xp.tile([128, XLEN], f32r, name=f"xpad_{n}_{cit}", tag="xpad")
                    # zero: slot 0 + top row | bottom row + trailing slot |
                    # shared pad column (flat 57(r+1) for data rows r=1..56)
                    nc.vector.tensor_copy(out=t[:, 0:58], in_=zrow[:])
                    nc.vector.tensor_copy(out=t[:, 1 + 57 * WR : XLEN], in_=zrow[:])
                    nc.vector.tensor_copy(
                        out=t[:, 2 * WR : 1 + 57 * WR : WR], in_=zrow[:, :56]
                    )
                    interior = (
                        t[:, 1 + WR : 1 + 57 * WR]
                        .rearrange("p (h w) -> p h w", w=WR)[:, :, :W]
                    )
                    nc.vector.tensor_copy(out=interior, in_=stage[:])
                    xt[(n, cit)] = t

            for _n in range(B_PER_CORE):
                load_image(_n)

            def rhs_ap(n, cit, h0, kh, kw):
                o = (h0 + kh) * WR + kw
                return xt[(n, cit)][:, o : o + NFREE]

            def emit_mms(n, ct, blk, ps):
                """18 accumulating matmuls into one PSUM bank."""
                h0 = blk * RB
                for idx, (cit, k) in enumerate(
                    [(cit, k) for cit in range(2) for k in range(9)]
                ):
                    kh, kw = divmod(k, 3)
                    nc.tensor.matmul(
                        ps[:],
                        lhsT=w_sb[:, ct * 2 + cit, k, :],
                        rhs=rhs_ap(n, cit, h0, kh, kw),
                        start=(idx == 0),
                        stop=(idx == 17),
                    )

            def drain(n, ct, blk, ps):
                y = yp.tile([128, RB * W], f32, name="y", tag="y")
                valid = ps.rearrange("p (h w) -> p h w", w=WR)[:, :, :W]
                nc.vector.tensor_copy(out=y[:], in_=valid)
                h0 = blk * RB
                nc.sync.dma_start(
                    out=out[n, ct * 128 : (ct + 1) * 128, h0 : h0 + RB, :],
                    in_=y[:],
                )

            first = True
            for n in range(B_PER_CORE):
                for ct in range(2):
                    if first:
                        # cin-tile-major over the whole group: 63 matmuls of
                        # runway on cin tile 0 while cin tile 1 finishes DMA
                        pss = [
                            pp.tile([128, NFREE], f32, name=f"ps{blk}", tag="ps")
                            for blk in range(NBLK)
                        ]
                        for idx, (cit, k) in enumerate(
                            [(c, k) for c in range(2) for k in range(9)]
                        ):
                            kh, kw = divmod(k, 3)
                            for blk in range(NBLK):
                                nc.tensor.matmul(
                                    pss[blk][:],
                                    lhsT=w_sb[:, ct * 2 + cit, k, :],
                                    rhs=rhs_ap(n, cit, blk * RB, kh, kw),
                                    start=(idx == 0),
                                    stop=(idx == 17),
                                )
                        for blk in range(NBLK):
                            drain(n, ct, blk, pss[blk])
                        first = False
                    else:
                        for blk in range(NBLK):
                            ps = pp.tile([128, NFREE], f32, name="ps", tag="ps")
                            emit_mms(n, ct, blk, ps)
                            drain(n, ct, blk, ps)
    nc.compile()
    return nc


def _get_nc():
    if "nc" not in _CACHED:
        _CACHED["nc"] = _build_nc()
    return _CACHED["nc"]


def _prep_weights(W_arr):
    Wb = np.sign(np.asarray(W_arr, dtype=np.float32))
    # [co, ci, kh, kw] -> [ct, cit, ci, k, co] -> [4, 128, 9, 128]
    wt = (
        Wb.reshape(2, 128, 2, 128, 9)
        .transpose(0, 2, 3, 4, 1)
        .reshape(4, 128, 9, 128)
    )
    return np.ascontiguousarray(wt)


def run(x, W, trace=False, trace_kwargs=None):
    from concourse.bass_utils import run_bass_kernel_spmd

    x = np.asarray(x, dtype=np.float32)
    wt = _prep_weights(W)
    nc = _get_nc()
    in_maps = [
        {"xs": np.ascontiguousarray(x[i * B_PER_CORE : (i + 1) * B_PER_CORE]), "wt": wt}
        for i in range(N_CORES)
    ]
    res = run_bass_kernel_spmd(
        nc,
        in_maps,
        list(range(N_CORES)),
        trace=trace,
        trace_kwargs=trace_kwargs or {},
    )
    out = np.concatenate([np.asarray(res.results[i]["out"]) for i in range(N_CORES)])
    return out, res


def kernel(x, W):
    out, _ = run(x, W, trace=False)
    return out


# revision 17
# speedup vs baseline: 1.0127x; 1.0127x over previous
"""Binary-weight 3x3 conv (stride 1, pad 1) on 8 TRN2 NeuronCores.

Strategy: data-parallel over batch (4 images per core), weights replicated.
Per image the conv is 9 shifted [Cin,Cout] matmuls accumulated in PSUM
with channels on the partition dim (NCHW layout already has x[n] as a
[C, H*W] channel-major matrix). The input lives in SBUF as fp32r rows of
width 57: data cols 0..55 plus one zero column that doubles as the next
row's LEFT pad, with zero rows above/below (flat [1 + 58*57 + 1] layout).
Every matmul rhs is then a fully CONTIGUOUS 1D window of N=456 covering
8 output rows (one junk psum column per row, discarded by the PSUM->SBUF
copy). Contiguous 1D rhs APs issue ~7ns/matmul faster than 3D strided
ones. fp32r (reduced-precision fp32 matmul mode) runs at 1 cycle/row on
the PE, 4x faster than plain fp32; binarized weights (+-1, 0) are exact.
Input is DMA'd contiguously into a staging tile (halves split across the
sync+scalar HWDGE queues) and padded/rounded into fp32r by a DVE copy.
Each PSUM bank gets its 18 accumulating matmuls back-to-back and drains
immediately; the first group runs cin-tile-major for a 63-matmul runway
while the second cin tile's DMA completes, and the first weight block is
DMA'd per-tap so tap 0 lands before the image does.
"""

import numpy as np

N_CORES = 8
B_PER_CORE = 4  # 32 images / 8 cores
CIN = 256
COUT = 256
H = W = 56
WR = 57  # row pitch: 56 data + 1 shared pad col
XLEN = 1 + 58 * WR + 1  # leading pad slot + 58 rows + trailing slot
RB = 8  # output rows per matmul
NBLK = H // RB  # 7
NFREE = RB * WR  # 456 (8 rows x 57, one junk col per row)

_CACHED = {}


def _build_nc():
    import concourse.mybir as mybir
    from concourse import bacc
    from concourse.tile import TileContext

    f32 = mybir.dt.float32
    f32r = mybir.dt.float32r

    nc = bacc.Bacc("TRN2", target_bir_lowering=False, debug=False)
    xs = nc.dram_tensor("xs", [B_PER_CORE, CIN, H, W], f32, kind="ExternalInput").ap()
    wt = nc.dram_tensor("wt", [4, 128, 9, 128], f32, kind="ExternalInput").ap()
    out = nc.dram_tensor(
        "out", [B_PER_CORE, COUT, H, W], f32, kind="ExternalOutput"
    ).ap()

    with TileContext(nc) as tc:
        with (
            tc.tile_pool(name="wp", bufs=1) as wp,
            tc.tile_pool(name="sp", bufs=2) as sp,
            tc.tile_pool(name="xp", bufs=8) as xp,
            tc.tile_pool(name="yp", bufs=16) as yp,
            tc.tile_pool(name="pp", bufs=8, space="PSUM") as pp,
        ):
            # fp32r memset is not a legal ISA op; round zeros in via DVE copy
            zrow = wp.tile([128, 58], f32, name="zrow")
            nc.vector.memset(zrow[:], 0.0)

            w_sb = wp.tile([128, 4, 9, 128], f32r, name="w_sb")

            HH = H // 2
            xt = {}

            stages = {}

            def load_image(n):
                for cit in range(2):
                    stage = sp.tile([128, H, W], f32, name="stage", tag="stage")
                    stages[(n, cit)] = stage
                    src = xs[n, cit * 128 : (cit + 1) * 128]
                    if n == 0 and cit == 0:
                        # first weight tap leads on scalar so the very first
                        # matmul is unblocked before the image lands
                        nc.scalar.dma_start(
                            out=w_sb[:, 0, 0], in_=wt[0, :, 0].bitcast(f32r)
                        )
                    nc.sync.dma_start(out=stage[:, :HH], in_=src[:, :HH])
                    nc.scalar.dma_start(out=stage[:, HH:], in_=src[:, HH:])
                    if n == 0 and cit == 0:
                        for k in range(1, 9):
                            nc.sync.dma_start(
                                out=w_sb[:, 0, k], in_=wt[0, :, k].bitcast(f32r)
                            )
                        nc.scalar.dma_start(out=w_sb[:, 1], in_=wt[1].bitcast(f32r))
                    if n == 0 and cit == 1:
                        nc.sync.dma_start(out=w_sb[:, 2], in_=wt[2].bitcast(f32r))
                        nc.scalar.dma_start(out=w_sb[:, 3], in_=wt[3].bitcast(f32r))
            def make_pad(n):
                # DVE is strict FIFO: emit pad-casts only when the images are
                # about to be needed, so early drain copies aren't queued
                # behind casts whose DMAs land late
                for cit in range(2):
                    t = xp.tile([128, XLEN], f32r, name=f"xpad_{n}_{cit}", tag="xpad")
                    # zero: slot 0 + top row | bottom row + trailing slot |
                    # shared pad column (flat 57(r+1) for data rows r=1..56)
                    nc.vector.tensor_copy(out=t[:, 0:58], in_=zrow[:])
                    nc.vector.tensor_copy(out=t[:, 1 + 57 * WR : XLEN], in_=zrow[:])
                    nc.vector.tensor_copy(
                        out=t[:, 2 * WR : 1 + 57 * WR : WR], in_=zrow[:, :56]
                    )
                    interior = (
                        t[:, 1 + WR : 1 + 57 * WR]
                        .rearrange("p (h w) -> p h w", w=WR)[:, :, :W]
                    )
                    nc.vector.tensor_copy(out=interior, in_=stages[(n, cit)][:])
                    xt[(n, cit)] = t

            for _n in range(B_PER_CORE):
                load_image(_n)
            make_pad(0)
            make_pad(1)

            def rhs_ap(n, cit, h0, kh, kw):
                o = (h0 + kh) * WR + kw
                return xt[(n, cit)][:, o : o + NFREE]

            def emit_mms(n, ct, blk, ps):
                """18 accumulating matmuls into one PSUM bank."""
                h0 = blk * RB
                for idx, (cit, k) in enumerate(
                    [(cit, k) for cit in range(2) for k in range(9)]
                ):
                    kh, kw = divmod(k, 3)
                    nc.tensor.matmul(
                        ps[:],
                        lhsT=w_sb[:, ct * 2 + cit, k, :],
                        rhs=rhs_ap(n, cit, h0, kh, kw),
                        start=(idx == 0),
                        stop=(idx == 17),
                    )

            def drain(n, ct, blk, ps):
                y = yp.tile([128, RB * W], f32, name="y", tag="y")
                valid = ps.rearrange("p (h w) -> p h w", w=WR)[:, :, :W]
                nc.vector.tensor_copy(out=y[:], in_=valid)
                h0 = blk * RB
                nc.sync.dma_start(
                    out=out[n, ct * 128 : (ct + 1) * 128, h0 : h0 + RB, :],
                    in_=y[:],
                )

            first = True
            for n in range(B_PER_CORE):
                for ct in range(2):
                    if first:
                        # cin-tile-major over the whole group: 63 matmuls of
                        # runway on cin tile 0 while cin tile 1 finishes DMA
                        pss = [
                            pp.tile([128, NFREE], f32, name=f"ps{blk}", tag="ps")
                            for blk in range(NBLK)
                        ]
                        for idx, (cit, k) in enumerate(
                            [(c, k) for c in range(2) for k in range(9)]
                        ):
                            kh, kw = divmod(k, 3)
                            for blk in range(NBLK):
                                nc.tensor.matmul(
                                    pss[blk][:],
                                    lhsT=w_sb[:, ct * 2 + cit, k, :],
                                    rhs=rhs_ap(n, cit, blk * RB, kh, kw),
                                    start=(idx == 0),
                                    stop=(idx == 17),
                                )
                        for blk in range(NBLK):
                            drain(n, ct, blk, pss[blk])
                        first = False
                        make_pad(2)
                    else:
                        for blk in range(NBLK):
                            ps = pp.tile([128, NFREE], f32, name="ps", tag="ps")
                            emit_mms(n, ct, blk, ps)
                            drain(n, ct, blk, ps)
                        if n == 0 and ct == 1:
                            make_pad(3)
    nc.compile()
    return nc


def _get_nc():
    if "nc" not in _CACHED:
        _CACHED["nc"] = _build_nc()
    return _CACHED["nc"]


def _prep_weights(W_arr):
    Wb = np.sign(np.asarray(W_arr, dtype=np.float32))
    # [co, ci, kh, kw] -> [ct, cit, ci, k, co] -> [4, 128, 9, 128]
    wt = (
        Wb.reshape(2, 128, 2, 128, 9)
        .transpose(0, 2, 3, 4, 1)
        .reshape(4, 128, 9, 128)
    )
    return np.ascontiguousarray(wt)


def run(x, W, trace=False, trace_kwargs=None):
    from concourse.bass_utils import run_bass_kernel_spmd

    x = np.asarray(x, dtype=np.float32)
    wt = _prep_weights(W)
    nc = _get_nc()
    in_maps = [
        {"xs": np.ascontiguousarray(x[i * B_PER_CORE : (i + 1) * B_PER_CORE]), "wt": wt}
        for i in range(N_CORES)
    ]
    res = run_bass_kernel_spmd(
        nc,
        in_maps,
        list(range(N_CORES)),
        trace=trace,
        trace_kwargs=trace_kwargs or {},
    )
    out = np.concatenate([np.asarray(res.results[i]["out"]) for i in range(N_CORES)])
    return out, res


def kernel(x, W):
    out, _ = run(x, W, trace=False)
    return out


# revision 18
# speedup vs baseline: 1.0314x; 1.0185x over previous
"""Binary-weight 3x3 conv (stride 1, pad 1) on 8 TRN2 NeuronCores.

Strategy: data-parallel over batch (4 images per core), weights replicated.
Per image the conv is 9 shifted [Cin,Cout] matmuls accumulated in PSUM
with channels on the partition dim (NCHW layout already has x[n] as a
[C, H*W] channel-major matrix). The input lives in SBUF as fp32r rows of
width 57: data cols 0..55 plus one zero column that doubles as the next
row's LEFT pad, with zero rows above/below (flat [1 + 58*57 + 1] layout).
Every matmul rhs is then a fully CONTIGUOUS 1D window of N=456 covering
8 output rows (one junk psum column per row, discarded by the PSUM->SBUF
copy). Contiguous 1D rhs APs issue ~7ns/matmul faster than 3D strided
ones. fp32r (reduced-precision fp32 matmul mode) runs at 1 cycle/row on
the PE, 4x faster than plain fp32; binarized weights (+-1, 0) are exact.
Input is DMA'd contiguously into a staging tile (halves split across the
sync+scalar HWDGE queues) and padded/rounded into fp32r by a DVE copy.
Each PSUM bank gets its 18 accumulating matmuls back-to-back and drains
immediately; the first group runs cin-tile-major for a 63-matmul runway
while the second cin tile's DMA completes, and the first weight block is
DMA'd per-tap so tap 0 lands before the image does.
"""

import numpy as np

N_CORES = 8
B_PER_CORE = 4  # 32 images / 8 cores
CIN = 256
COUT = 256
H = W = 56
WR = 57  # row pitch: 56 data + 1 shared pad col
XLEN = 1 + 58 * WR + 1  # leading pad slot + 58 rows + trailing slot
RB = 8  # output rows per matmul
NBLK = H // RB  # 7
NFREE = RB * WR  # 456 (8 rows x 57, one junk col per row)

_CACHED = {}


def _build_nc():
    import concourse.mybir as mybir
    from concourse import bacc
    from concourse.tile import TileContext

    f32 = mybir.dt.float32
    f32r = mybir.dt.float32r

    nc = bacc.Bacc("TRN2", target_bir_lowering=False, debug=False)
    xs = nc.dram_tensor("xs", [B_PER_CORE, CIN, H, W], f32, kind="ExternalInput").ap()
    wt = nc.dram_tensor("wt", [4, 128, 9, 128], f32, kind="ExternalInput").ap()
    out = nc.dram_tensor(
        "out", [B_PER_CORE, COUT, H, W], f32, kind="ExternalOutput"
    ).ap()

    with TileContext(nc) as tc:
        with (
            tc.tile_pool(name="wp", bufs=1) as wp,
            tc.tile_pool(name="sp", bufs=2) as sp,
            tc.tile_pool(name="xp", bufs=8) as xp,
            tc.tile_pool(name="yp", bufs=16) as yp,
            tc.tile_pool(name="pp", bufs=8, space="PSUM") as pp,
        ):
            # fp32r memset is not a legal ISA op; round zeros in via DVE copy
            zrow = wp.tile([128, 58], f32, name="zrow")
            nc.vector.memset(zrow[:], 0.0)

            w_sb = wp.tile([128, 4, 9, 128], f32r, name="w_sb")

            HH = H // 2
            xt = {}

            stages = {}

            def load_image(n):
                for cit in range(2):
                    stage = sp.tile([128, H, W], f32, name="stage", tag="stage")
                    stages[(n, cit)] = stage
                    src = xs[n, cit * 128 : (cit + 1) * 128]
                    if n == 0 and cit == 0:
                        # first weight tap leads on scalar so the very first
                        # matmul is unblocked before the image lands
                        nc.scalar.dma_start(
                            out=w_sb[:, 0, 0], in_=wt[0, :, 0].bitcast(f32r)
                        )
                    nc.sync.dma_start(out=stage[:, :HH], in_=src[:, :HH])
                    nc.scalar.dma_start(out=stage[:, HH:], in_=src[:, HH:])
                    if n == 0 and cit == 0:
                        for k in range(1, 9):
                            nc.sync.dma_start(
                                out=w_sb[:, 0, k], in_=wt[0, :, k].bitcast(f32r)
                            )
                        nc.scalar.dma_start(out=w_sb[:, 1], in_=wt[1].bitcast(f32r))
                    if n == 0 and cit == 1:
                        nc.sync.dma_start(out=w_sb[:, 2], in_=wt[2].bitcast(f32r))
                        nc.scalar.dma_start(out=w_sb[:, 3], in_=wt[3].bitcast(f32r))
            def make_pad(n):
                # DVE is strict FIFO: emit pad-casts only when the images are
                # about to be needed, so early drain copies aren't queued
                # behind casts whose DMAs land late
                for cit in range(2):
                    t = xp.tile([128, XLEN], f32r, name=f"xpad_{n}_{cit}", tag="xpad")
                    # zero: slot 0 + top row | bottom row + trailing slot |
                    # shared pad column (flat 57(r+1) for data rows r=1..56)
                    nc.vector.tensor_copy(out=t[:, 0:58], in_=zrow[:])
                    nc.vector.tensor_copy(out=t[:, 1 + 57 * WR : XLEN], in_=zrow[:])
                    nc.vector.tensor_copy(
                        out=t[:, 2 * WR : 1 + 57 * WR : WR], in_=zrow[:, :56]
                    )
                    interior = (
                        t[:, 1 + WR : 1 + 57 * WR]
                        .rearrange("p (h w) -> p h w", w=WR)[:, :, :W]
                    )
                    nc.vector.tensor_copy(out=interior, in_=stages[(n, cit)][:])
                    xt[(n, cit)] = t

            for _n in range(B_PER_CORE):
                load_image(_n)
            make_pad(0)
            make_pad(1)

            def rhs_ap(n, cit, h0, kh, kw):
                o = (h0 + kh) * WR + kw
                return xt[(n, cit)][:, o : o + NFREE]

            def emit_mms(n, ct, blk, ps):
                """18 accumulating matmuls into one PSUM bank."""
                h0 = blk * RB
                for idx, (cit, k) in enumerate(
                    [(cit, k) for cit in range(2) for k in range(9)]
                ):
                    kh, kw = divmod(k, 3)
                    nc.tensor.matmul(
                        ps[:],
                        lhsT=w_sb[:, ct * 2 + cit, k, :],
                        rhs=rhs_ap(n, cit, h0, kh, kw),
                        start=(idx == 0),
                        stop=(idx == 17),
                    )

            def drain(n, ct, blk, ps):
                y = yp.tile([128, RB * W], f32, name="y", tag="y")
                valid = ps.rearrange("p (h w) -> p h w", w=WR)[:, :, :W]
                nc.vector.tensor_copy(out=y[:], in_=valid)
                h0 = blk * RB
                nc.sync.dma_start(
                    out=out[n, ct * 128 : (ct + 1) * 128, h0 : h0 + RB, :],
                    in_=y[:],
                )

            first = True
            for n in range(B_PER_CORE):
                for ct in range(2):
                    if first:
                        # cin-tile-major over the whole group: 63 matmuls of
                        # runway on cin tile 0 while cin tile 1 finishes DMA
                        pss = [
                            pp.tile([128, NFREE], f32, name=f"ps{blk}", tag="ps")
                            for blk in range(NBLK)
                        ]
                        for idx, (cit, k) in enumerate(
                            [(c, k) for c in range(2) for k in range(9)]
                        ):
                            kh, kw = divmod(k, 3)
                            for blk in range(NBLK):
                                nc.tensor.matmul(
                                    pss[blk][:],
                                    lhsT=w_sb[:, ct * 2 + cit, k, :],
                                    rhs=rhs_ap(n, cit, blk * RB, kh, kw),
                                    start=(idx == 0),
                                    stop=(idx == 17),
                                )
                        for blk in range(NBLK):
                            drain(n, ct, blk, pss[blk])
                        first = False
                    else:
                        for blk in range(NBLK):
                            ps = pp.tile([128, NFREE], f32, name="ps", tag="ps")
                            emit_mms(n, ct, blk, ps)
                            drain(n, ct, blk, ps)
                        if n == 1 and ct == 0:
                            make_pad(2)
                        if n == 1 and ct == 1:
                            make_pad(3)
    nc.compile()
    return nc


def _get_nc():
    if "nc" not in _CACHED:
        _CACHED["nc"] = _build_nc()
    return _CACHED["nc"]


def _prep_weights(W_arr):
    Wb = np.sign(np.asarray(W_arr, dtype=np.float32))
    # [co, ci, kh, kw] -> [ct, cit, ci, k, co] -> [4, 128, 9, 128]
    wt = (
        Wb.reshape(2, 128, 2, 128, 9)
        .transpose(0, 2, 3, 4, 1)
        .reshape(4, 128, 9, 128)
    )
    return np.ascontiguousarray(wt)


def run(x, W, trace=False, trace_kwargs=None):
    from concourse.bass_utils import run_bass_kernel_spmd

    x = np.asarray(x, dtype=np.float32)
    wt = _prep_weights(W)
    nc = _get_nc()
    in_maps = [
        {"xs": np.ascontiguousarray(x[i * B_PER_CORE : (i + 1) * B_PER_CORE]), "wt": wt}
        for i in range(N_CORES)
    ]
    res = run_bass_kernel_spmd(
        nc,
        in_maps,
        list(range(N_CORES)),
        trace=trace,
        trace_kwargs=trace_kwargs or {},
    )
    out = np.concatenate([np.asarray(res.results[i]["out"]) for i in range(N_CORES)])
    return out, res


def kernel(x, W):
    out, _ = run(x, W, trace=False)
    return out
